# revision 3
# baseline (speedup 1.0000x reference)
"""Trainium2 Bass kernel for nn_Encoder (attention-augmented LSTM encoder).

Strategy: data-parallel over batch across 8 NeuronCores (64 rows each).
Per core the T-1=99 sequential recurrence runs with 2 software-pipelined
"waves" of 32 batch rows so all engines stay busy despite the serial
dependence. The big per-step tensor tanh(z1[b,s] + z2[b,n,s]) lives as
[s=99 partitions, (b,n) free] in bf16; its s-contraction with w3 runs as
32 per-batch-row stationary matmuls on the PE producing e^T across 128
partitions. Sigmoids are computed as 0.5*(1+tanh(x/2)) so the whole
kernel needs a single ACT table set (exp_and_others: exp + tanh).
"""
import sys
import numpy as np

sys.path.insert(0, "/opt/trn_rl_repo")

B_FULL, TT, N, H = 512, 99, 128, 256
NCORES = 8
BSH = B_FULL // NCORES          # 64 batch rows per core
WAVES = 2
BW = BSH // WAVES               # 32 rows per wave
G4 = 4 * H                      # 1024

_cache = {}


def _gate_perm():
    # pytorch gate order i,f,g,o -> our column order [i, f, o, g]
    idx = np.arange(G4)
    return np.concatenate([idx[0:H], idx[H:2*H], idx[3*H:4*H], idx[2*H:3*H]])


def _build_program(TT_run=TT):
    import concourse.bacc as bacc
    import concourse.mybir as mybir
    from concourse.tile import TileContext

    dt = mybir.dt
    AF = mybir.ActivationFunctionType
    ALU = mybir.AluOpType

    nc = bacc.Bacc("TRN2", target_bir_lowering=False, debug=False,
                   num_devices=NCORES)

    f32, bf16 = dt.float32, dt.bfloat16

    x_d = nc.dram_tensor("x", [BSH, TT, N], f32, kind="ExternalInput")
    w1t_d = nc.dram_tensor("w1t", [128, 4 * TT], bf16, kind="ExternalInput")
    w2t_d = nc.dram_tensor("w2t", [TT, TT], bf16, kind="ExternalInput")
    w3_d = nc.dram_tensor("w3", [TT, 1], bf16, kind="ExternalInput")
    biass_d = nc.dram_tensor("biass", [TT, 1], f32, kind="ExternalInput")
    wih_d = nc.dram_tensor("wih", [N, G4], bf16, kind="ExternalInput")
    whh_d = nc.dram_tensor("whh", [128, 2 * G4], bf16, kind="ExternalInput")
    brow_d = nc.dram_tensor("brow", [1, G4], bf16, kind="ExternalInput")
    ones_d = nc.dram_tensor("ones1", [1, BW], bf16, kind="ExternalInput")
    ident_d = nc.dram_tensor("ident", [128, 128], f32, kind="ExternalInput")
    wout_d = nc.dram_tensor("wout", [BSH, TT, N], f32, kind="ExternalOutput")
    hout_d = nc.dram_tensor("hout", [BSH, TT, H], f32, kind="ExternalOutput")

    with TileContext(nc) as tc:
        with tc.tile_pool(name="const", bufs=1) as cp:
            w1t_sb = cp.tile([128, 4 * TT], bf16)
            nc.sync.dma_start(out=w1t_sb, in_=w1t_d[:])
            w2t_sb = cp.tile([TT, TT], bf16)
            nc.sync.dma_start(out=w2t_sb, in_=w2t_d[:])
            w3_sb = cp.tile([TT, 1], bf16)
            nc.sync.dma_start(out=w3_sb, in_=w3_d[:])
            biass_sb = cp.tile([TT, 1], f32)
            nc.sync.dma_start(out=biass_sb, in_=biass_d[:])
            wih_sb = cp.tile([N, G4], bf16)
            nc.sync.dma_start(out=wih_sb, in_=wih_d[:])
            whh_sb = cp.tile([128, 2 * G4], bf16)
            nc.sync.dma_start(out=whh_sb, in_=whh_d[:])
            brow_sb = cp.tile([1, G4], bf16)
            nc.sync.dma_start(out=brow_sb, in_=brow_d[:])
            ones_sb = cp.tile([1, BW], bf16)
            nc.sync.dma_start(out=ones_sb, in_=ones_d[:])
            ident_sb = cp.tile([128, 128], f32)
            nc.sync.dma_start(out=ident_sb, in_=ident_d[:])
            z2_sb = cp.tile([TT, BSH * N], bf16)  # [s, b, n] b-outer

            # ---- z2 precompute: z2[s,b,n] = sum_t x[b,t,n] W2[s,t] + b1[s]+b2[s]
            with tc.tile_pool(name="pre", bufs=2) as pp, \
                 tc.tile_pool(name="prep", bufs=2, space="PSUM") as ppp:
                xT_sb = pp.tile([TT, BSH * N], f32, bufs=1)
                nc.sync.dma_start(out=xT_sb,
                                  in_=x_d[:].rearrange("b t n -> t b n"))
                xT_bf = pp.tile([TT, BSH * N], bf16, bufs=1)
                nc.vector.tensor_copy(xT_bf, xT_sb)
                CH = 2048
                for c in range(0, BSH * N, CH):
                    zc_ps = ppp.tile([TT, CH], f32, tag="zc")
                    for cc in range(0, CH, 512):
                        nc.tensor.matmul(zc_ps[:, cc:cc + 512],
                                         lhsT=w2t_sb,
                                         rhs=xT_bf[:, c + cc:c + cc + 512],
                                         start=True, stop=True)
                    nc.scalar.activation(z2_sb[:, c:c + CH], zc_ps,
                                         AF.Identity, bias=biass_sb)

            # ---- main recurrence
            with tc.tile_pool(name="big", bufs=2) as bigp, \
                 tc.tile_pool(name="sm", bufs=3) as smp, \
                 tc.tile_pool(name="st", bufs=2) as stp, \
                 tc.tile_pool(name="xtp", bufs=6) as xtp, \
                 tc.tile_pool(name="outp", bufs=3) as outp, \
                 tc.tile_pool(name="pz1", bufs=1, space="PSUM") as pz1, \
                 tc.tile_pool(name="peT", bufs=1, space="PSUM") as peT, \
                 tc.tile_pool(name="pe", bufs=1, space="PSUM") as pe, \
                 tc.tile_pool(name="ptr", bufs=3, space="PSUM") as ptr, \
                 tc.tile_pool(name="pg", bufs=1, space="PSUM") as pg:

                # initial zero state per wave
                state = []
                for w in range(WAVES):
                    hT0 = stp.tile([128, BW], bf16, tag=f"hT0_{w}")
                    hT1 = stp.tile([128, BW], bf16, tag=f"hT1_{w}")
                    cT0 = stp.tile([128, BW], bf16, tag=f"cT0_{w}")
                    cT1 = stp.tile([128, BW], bf16, tag=f"cT1_{w}")
                    Ch = stp.tile([BW, H], f32, tag=f"Ch_{w}")
                    for tl in (hT0, hT1, cT0, cT1, Ch):
                        nc.vector.memset(tl, 0.0)
                    state.append({"hT0": hT0, "hT1": hT1, "cT0": cT0,
                                  "cT1": cT1, "Ch": Ch})

                z2v = z2_sb.rearrange("s (b n) -> s b n", b=BSH)

                for t in range(TT_run):
                    for w in range(WAVES):
                        st_ = state[w]
                        b0 = w * BW
                        # x_t prefetch
                        xt_sb = xtp.tile([BW, N], f32, tag=f"xt_{w}")
                        nc.sync.dma_start(out=xt_sb,
                                          in_=x_d[b0:b0 + BW, t, :])

                        # z1T = W1T.T @ [h2T; cT]  -> psum [99, BW]
                        z1_ps = pz1.tile([TT, BW], f32, tag="z1")
                        hc = (st_["hT0"], st_["hT1"], st_["cT0"], st_["cT1"])
                        for k in range(4):
                            nc.tensor.matmul(
                                z1_ps, lhsT=w1t_sb[:, k * TT:(k + 1) * TT],
                                rhs=hc[k], start=(k == 0), stop=(k == 3))
                        z1_bf = smp.tile([TT, BW], bf16, tag=f"z1bf_{w}")
                        nc.vector.tensor_copy(z1_bf, z1_ps)

                        # A = z2[:, wave, :] + bcast_n(z1)  (bf16)
                        A_bf = bigp.tile([TT, BW * N], bf16, tag=f"A_{w}")
                        nc.vector.tensor_tensor(
                            out=A_bf.rearrange("s (b n) -> s b n", b=BW),
                            in0=z2v[:, b0:b0 + BW, :],
                            in1=z1_bf.unsqueeze(2).broadcast_to([TT, BW, N]),
                            op=ALU.add)

                        # X = tanh(A)
                        X_bf = bigp.tile([TT, BW * N], bf16, tag=f"X_{w}")
                        nc.scalar.activation(X_bf, A_bf, AF.Tanh)

                        # eT[n, b] = sum_s w3[s] X[s, b, n]
                        eT_ps = peT.tile([N, BW], f32, tag="eT")
                        Xv = X_bf.rearrange("s (b n) -> s b n", b=BW)
                        for b in range(BW):
                            nc.tensor.matmul(eT_ps[:, b:b + 1],
                                             lhsT=Xv[:, b, :], rhs=w3_sb,
                                             start=True, stop=True)
                        eT_sb = smp.tile([N, BW], f32, tag=f"eT_{w}")
                        nc.vector.tensor_copy(eT_sb, eT_ps)

                        # e = transpose(eT)
                        e_ps = pe.tile([BW, N], f32, tag="e")
                        nc.tensor.transpose(e_ps, eT_sb, ident_sb)

                        # softmax (no max subtraction; |e| <~ 4) + x_t mult
                        P_sb = smp.tile([BW, N], f32, tag=f"P_{w}")
                        S_sb = smp.tile([BW, 1], f32, tag=f"S_{w}")
                        nc.scalar.activation(P_sb, e_ps, AF.Exp,
                                             accum_out=S_sb)
                        r_sb = smp.tile([BW, 1], f32, tag=f"r_{w}")
                        nc.vector.reciprocal(r_sb, S_sb)
                        w_sb = outp.tile([BW, N], f32, tag=f"w_{w}")
                        nc.vector.scalar_tensor_tensor(
                            out=w_sb, in0=P_sb, scalar=r_sb, in1=xt_sb,
                            op0=ALU.mult, op1=ALU.mult)
                        nc.sync.dma_start(out=wout_d[b0:b0 + BW, t, :],
                                          in_=w_sb)

                        # wT (bf16) for gates
                        wT_ps = ptr.tile([N, BW], f32, tag="tr")
                        nc.tensor.transpose(wT_ps, w_sb, ident_sb[0:BW, 0:BW])
                        wT_bf = smp.tile([N, BW], bf16, tag=f"wT_{w}")
                        nc.vector.tensor_copy(wT_bf, wT_ps)

                        # gates = wT.T@Wih + h2T.T@(Whh/2) + bias  [BW, 1024]
                        g_ps = pg.tile([BW, G4], f32, tag="g")
                        for q in range(0, G4, 512):
                            nc.tensor.matmul(g_ps[:, q:q + 512], lhsT=wT_bf,
                                             rhs=wih_sb[:, q:q + 512],
                                             start=True, stop=False)
                            nc.tensor.matmul(g_ps[:, q:q + 512],
                                             lhsT=st_["hT0"],
                                             rhs=whh_sb[:, q:q + 512],
                                             start=False, stop=False)
                            nc.tensor.matmul(g_ps[:, q:q + 512],
                                             lhsT=st_["hT1"],
                                             rhs=whh_sb[:, G4 + q:G4 + q + 512],
                                             start=False, stop=False)
                            nc.tensor.matmul(g_ps[:, q:q + 512], lhsT=ones_sb,
                                             rhs=brow_sb[:, q:q + 512],
                                             start=False, stop=True)

                        # LSTM with tanh-only nonlinearity (cols: i,f,o,g)
                        Tifo = smp.tile([BW, 3 * H], f32, tag=f"Tifo_{w}")
                        nc.scalar.activation(Tifo, g_ps[:, 0:3 * H], AF.Tanh,
                                             scale=0.5)
                        Tg = smp.tile([BW, H], f32, tag=f"Tg_{w}")
                        nc.scalar.activation(Tg, g_ps[:, 3 * H:4 * H], AF.Tanh)

                        u_sb = smp.tile([BW, H], f32, tag=f"u_{w}")
                        nc.vector.scalar_tensor_tensor(
                            out=u_sb, in0=Tifo[:, 0:H], scalar=1.0, in1=Tg,
                            op0=ALU.add, op1=ALU.mult)
                        v_sb = smp.tile([BW, H], f32, tag=f"v_{w}")
                        nc.vector.scalar_tensor_tensor(
                            out=v_sb, in0=Tifo[:, H:2 * H], scalar=1.0,
                            in1=st_["Ch"], op0=ALU.add, op1=ALU.mult)
                        c_sb = smp.tile([BW, H], f32, tag=f"c_{w}")
                        nc.vector.scalar_tensor_tensor(
                            out=c_sb, in0=u_sb, scalar=0.5, in1=v_sb,
                            op0=ALU.mult, op1=ALU.add)
                        Ch_new = stp.tile([BW, H], f32, tag=f"Ch_{w}")
                        nc.vector.tensor_scalar_mul(Ch_new, c_sb, 0.5)
                        Tc = smp.tile([BW, H], f32, tag=f"Tc_{w}")
                        nc.scalar.activation(Tc, c_sb, AF.Tanh)
                        h2_sb = smp.tile([BW, H], f32, tag=f"h2_{w}")
                        nc.vector.scalar_tensor_tensor(
                            out=h2_sb, in0=Tifo[:, 2 * H:3 * H], scalar=1.0,
                            in1=Tc, op0=ALU.add, op1=ALU.mult)
                        h_sb = outp.tile([BW, H], f32, tag=f"h_{w}")
                        nc.vector.tensor_scalar_mul(h_sb, h2_sb, 0.5)
                        nc.sync.dma_start(out=hout_d[b0:b0 + BW, t, :],
                                          in_=h_sb)

                        # transposes for next step state (h2T, cT) in bf16
                        new_st = {"Ch": Ch_new}
                        for nm, src in (("hT0", h2_sb[:, 0:128]),
                                        ("hT1", h2_sb[:, 128:256]),
                                        ("cT0", c_sb[:, 0:128]),
                                        ("cT1", c_sb[:, 128:256])):
                            tr_ps = ptr.tile([128, BW], f32, tag="tr")
                            nc.tensor.transpose(tr_ps, src,
                                                ident_sb[0:BW, 0:BW])
                            tr_bf = stp.tile([128, BW], bf16, tag=f"{nm}_{w}")
                            nc.vector.tensor_copy(tr_bf, tr_ps)
                            new_st[nm] = tr_bf
                        state[w] = new_st

    nc.finalize()
    return nc


def _prep_host_inputs(inputs):
    import ml_dtypes
    bf = ml_dtypes.bfloat16
    W1, b1 = np.asarray(inputs["W1"]), np.asarray(inputs["b1"])
    W2, b2 = np.asarray(inputs["W2"]), np.asarray(inputs["b2"])
    W3 = np.asarray(inputs["W3"])
    W_ih, W_hh = np.asarray(inputs["W_ih"]), np.asarray(inputs["W_hh"])
    b_ih, b_hh = np.asarray(inputs["b_ih"]), np.asarray(inputs["b_hh"])

    perm = _gate_perm()
    # z1 uses [h2=2h, c] so scale W1's h-columns by 0.5
    W1s = W1.copy()
    W1s[:, 0:H] *= 0.5  # columns 0:H=256 are the h dims of [h, c]
    W1T = W1s.T.astype(np.float32)          # [512, 99]
    w1t = W1T.reshape(4, 128, TT).transpose(1, 0, 2).reshape(128, 4 * TT)

    whh_half = (0.5 * W_hh.T)[:, perm]      # [256, 1024]
    whh = whh_half.reshape(2, 128, G4).transpose(1, 0, 2).reshape(128, 2 * G4)

    return {
        "w1t": w1t.astype(bf),
        "w2t": W2.T.astype(np.float32).astype(bf),      # [t, s] = W2[s,t].T
        "w3": W3[0].reshape(TT, 1).astype(bf),
        "biass": (b1 + b2).reshape(TT, 1).astype(np.float32),
        "wih": W_ih.T[:, perm].astype(bf),              # [128, 1024]
        "whh": whh.astype(bf),
        "brow": (b_ih + b_hh)[perm].reshape(1, G4).astype(bf),
        "ones1": np.ones((1, BW), dtype=bf),
        "ident": np.eye(128, dtype=np.float32),
    }


def kernel(**inputs):
    from concourse.bass_utils import run_bass_kernel_spmd

    if "nc" not in _cache:
        _cache["nc"] = _build_program()
    nc = _cache["nc"]

    shared = _prep_host_inputs(inputs)
    x = np.ascontiguousarray(np.asarray(inputs["input_data"], dtype=np.float32))
    in_maps = []
    for c in range(NCORES):
        m = dict(shared)
        m["x"] = x[c * BSH:(c + 1) * BSH]
        in_maps.append(m)

    res = run_bass_kernel_spmd(nc, in_maps, core_ids=list(range(NCORES)))
    w_full = np.concatenate([r["wout"] for r in res.results], axis=0)
    h_full = np.concatenate([r["hout"] for r in res.results], axis=0)
    return w_full, h_full


# revision 5
# speedup vs baseline: 4818.7418x; 4818.7418x over previous
"""Trainium2 Bass kernel for nn_Encoder (attention-augmented LSTM encoder).

Strategy: data-parallel over batch across 8 NeuronCores (64 rows each).
Per core the T-1=99 sequential recurrence runs with 2 software-pipelined
"waves" of 32 batch rows so all engines stay busy despite the serial
dependence. The big per-step tensor tanh(z1[b,s] + z2[b,n,s]) lives as
[s=99 partitions, (b,n) free] in bf16; its s-contraction with w3 runs as
32 per-batch-row stationary matmuls on the PE producing e^T across 128
partitions. Sigmoids are computed as 0.5*(1+tanh(x/2)) so the whole
kernel needs a single ACT table set (exp_and_others: exp + tanh).
"""
import sys
import numpy as np

sys.path.insert(0, "/opt/trn_rl_repo")

B_FULL, TT, N, H = 512, 99, 128, 256
NCORES = 8
BSH = B_FULL // NCORES          # 64 batch rows per core
WAVES = 2
BW = BSH // WAVES               # 32 rows per wave
G4 = 4 * H                      # 1024

_cache = {}


def _gate_perm():
    # pytorch gate order i,f,g,o -> our column order [i, f, o, g]
    idx = np.arange(G4)
    return np.concatenate([idx[0:H], idx[H:2*H], idx[3*H:4*H], idx[2*H:3*H]])


def _build_program(TT_run=TT):
    import concourse.bacc as bacc
    import concourse.mybir as mybir
    from concourse.tile import TileContext

    dt = mybir.dt
    AF = mybir.ActivationFunctionType
    ALU = mybir.AluOpType

    nc = bacc.Bacc("TRN2", target_bir_lowering=False, debug=False,
                   num_devices=NCORES)

    f32, bf16 = dt.float32, dt.bfloat16

    x_d = nc.dram_tensor("x", [BSH, TT, N], f32, kind="ExternalInput")
    w1t_d = nc.dram_tensor("w1t", [128, 4 * TT], bf16, kind="ExternalInput")
    w2t_d = nc.dram_tensor("w2t", [TT, TT], bf16, kind="ExternalInput")
    w3_d = nc.dram_tensor("w3", [TT, 1], bf16, kind="ExternalInput")
    biass_d = nc.dram_tensor("biass", [TT, 1], f32, kind="ExternalInput")
    wih_d = nc.dram_tensor("wih", [N, G4], bf16, kind="ExternalInput")
    whh_d = nc.dram_tensor("whh", [128, 2 * G4], bf16, kind="ExternalInput")
    brow_d = nc.dram_tensor("brow", [1, G4], bf16, kind="ExternalInput")
    ones_d = nc.dram_tensor("ones1", [1, BW], bf16, kind="ExternalInput")
    ident_d = nc.dram_tensor("ident", [128, 128], f32, kind="ExternalInput")
    wout_d = nc.dram_tensor("wout", [BSH, TT, N], f32, kind="ExternalOutput")
    hout_d = nc.dram_tensor("hout", [BSH, TT, H], f32, kind="ExternalOutput")

    with TileContext(nc) as tc:
        with tc.tile_pool(name="const", bufs=1) as cp:
            w1t_sb = cp.tile([128, 4 * TT], bf16)
            nc.sync.dma_start(out=w1t_sb, in_=w1t_d[:])
            w2t_sb = cp.tile([TT, TT], bf16)
            nc.sync.dma_start(out=w2t_sb, in_=w2t_d[:])
            w3_sb = cp.tile([TT, 1], bf16)
            nc.sync.dma_start(out=w3_sb, in_=w3_d[:])
            biass_sb = cp.tile([TT, 1], f32)
            nc.sync.dma_start(out=biass_sb, in_=biass_d[:])
            wih_sb = cp.tile([N, G4], bf16)
            nc.sync.dma_start(out=wih_sb, in_=wih_d[:])
            whh_sb = cp.tile([128, 2 * G4], bf16)
            nc.sync.dma_start(out=whh_sb, in_=whh_d[:])
            brow_sb = cp.tile([1, G4], bf16)
            nc.sync.dma_start(out=brow_sb, in_=brow_d[:])
            ones_sb = cp.tile([1, BW], bf16)
            nc.sync.dma_start(out=ones_sb, in_=ones_d[:])
            ident_sb = cp.tile([128, 128], f32)
            nc.sync.dma_start(out=ident_sb, in_=ident_d[:])
            z2_sb = cp.tile([TT, BSH * N], bf16)  # [s, b, n] b-outer

            # ---- z2 precompute: z2[s,b,n] = sum_t x[b,t,n] W2[s,t] + b1[s]+b2[s]
            with tc.tile_pool(name="pre", bufs=2) as pp, \
                 tc.tile_pool(name="prep", bufs=2, space="PSUM") as ppp:
                xT_sb = pp.tile([TT, BSH * N], f32, bufs=1)
                nc.sync.dma_start(out=xT_sb,
                                  in_=x_d[:].rearrange("b t n -> t b n"))
                xT_bf = pp.tile([TT, BSH * N], bf16, bufs=1)
                nc.vector.tensor_copy(xT_bf, xT_sb)
                CH = 2048
                for c in range(0, BSH * N, CH):
                    zc_ps = ppp.tile([TT, CH], f32, tag="zc")
                    for cc in range(0, CH, 512):
                        nc.tensor.matmul(zc_ps[:, cc:cc + 512],
                                         lhsT=w2t_sb,
                                         rhs=xT_bf[:, c + cc:c + cc + 512],
                                         start=True, stop=True)
                    nc.scalar.activation(z2_sb[:, c:c + CH], zc_ps,
                                         AF.Identity, bias=biass_sb)

            # ---- main recurrence
            with tc.tile_pool(name="big", bufs=2) as bigp, \
                 tc.tile_pool(name="sm", bufs=3) as smp, \
                 tc.tile_pool(name="st", bufs=2) as stp, \
                 tc.tile_pool(name="xtp", bufs=6) as xtp, \
                 tc.tile_pool(name="outp", bufs=3) as outp, \
                 tc.tile_pool(name="pz1", bufs=1, space="PSUM") as pz1, \
                 tc.tile_pool(name="peT", bufs=1, space="PSUM") as peT, \
                 tc.tile_pool(name="pe", bufs=1, space="PSUM") as pe, \
                 tc.tile_pool(name="ptr", bufs=3, space="PSUM") as ptr, \
                 tc.tile_pool(name="pg", bufs=1, space="PSUM") as pg:

                # initial zero state per wave
                state = []
                for w in range(WAVES):
                    hT0 = stp.tile([128, BW], bf16, tag=f"hT0_{w}")
                    hT1 = stp.tile([128, BW], bf16, tag=f"hT1_{w}")
                    cT0 = stp.tile([128, BW], bf16, tag=f"cT0_{w}")
                    cT1 = stp.tile([128, BW], bf16, tag=f"cT1_{w}")
                    Ch = stp.tile([BW, H], f32, tag=f"Ch_{w}")
                    for tl in (hT0, hT1, cT0, cT1, Ch):
                        nc.vector.memset(tl, 0.0)
                    state.append({"hT0": hT0, "hT1": hT1, "cT0": cT0,
                                  "cT1": cT1, "Ch": Ch})

                z2v = z2_sb.rearrange("s (b n) -> s b n", b=BSH)

                for t in range(TT_run):
                    for w in range(WAVES):
                        st_ = state[w]
                        b0 = w * BW
                        # x_t prefetch
                        xt_sb = xtp.tile([BW, N], f32, tag=f"xt_{w}")
                        nc.sync.dma_start(out=xt_sb,
                                          in_=x_d[b0:b0 + BW, t, :])

                        # z1T = W1T.T @ [h2T; cT]  -> psum [99, BW]
                        z1_ps = pz1.tile([TT, BW], f32, tag="z1")
                        hc = (st_["hT0"], st_["hT1"], st_["cT0"], st_["cT1"])
                        for k in range(4):
                            nc.tensor.matmul(
                                z1_ps, lhsT=w1t_sb[:, k * TT:(k + 1) * TT],
                                rhs=hc[k], start=(k == 0), stop=(k == 3))
                        z1_bf = smp.tile([TT, BW], bf16, tag=f"z1bf_{w}")
                        nc.vector.tensor_copy(z1_bf, z1_ps)

                        # A = z2[:, wave, :] + bcast_n(z1)  (bf16)
                        A_bf = bigp.tile([TT, BW * N], bf16, tag=f"A_{w}")
                        nc.vector.tensor_tensor(
                            out=A_bf.rearrange("s (b n) -> s b n", b=BW),
                            in0=z2v[:, b0:b0 + BW, :],
                            in1=z1_bf.unsqueeze(2).broadcast_to([TT, BW, N]),
                            op=ALU.add)

                        # X = tanh(A)
                        X_bf = bigp.tile([TT, BW * N], bf16, tag=f"X_{w}")
                        nc.scalar.activation(X_bf, A_bf, AF.Tanh)

                        # eT[n, b] = sum_s w3[s] X[s, b, n]
                        eT_ps = peT.tile([N, BW], f32, tag="eT")
                        Xv = X_bf.rearrange("s (b n) -> s b n", b=BW)
                        for b in range(BW):
                            nc.tensor.matmul(eT_ps[:, b:b + 1],
                                             lhsT=Xv[:, b, :], rhs=w3_sb,
                                             start=True, stop=True)
                        eT_sb = smp.tile([N, BW], f32, tag=f"eT_{w}")
                        nc.vector.tensor_copy(eT_sb, eT_ps)

                        # e = transpose(eT)
                        e_ps = pe.tile([BW, N], f32, tag="e")
                        nc.tensor.transpose(e_ps, eT_sb, ident_sb)

                        # softmax (no max subtraction; |e| <~ 4) + x_t mult
                        P_sb = smp.tile([BW, N], f32, tag=f"P_{w}")
                        S_sb = smp.tile([BW, 1], f32, tag=f"S_{w}")
                        nc.scalar.activation(P_sb, e_ps, AF.Exp,
                                             accum_out=S_sb)
                        r_sb = smp.tile([BW, 1], f32, tag=f"r_{w}")
                        nc.vector.reciprocal(r_sb, S_sb)
                        w_sb = outp.tile([BW, N], f32, tag=f"w_{w}")
                        nc.vector.scalar_tensor_tensor(
                            out=w_sb, in0=P_sb, scalar=r_sb, in1=xt_sb,
                            op0=ALU.mult, op1=ALU.mult)
                        nc.sync.dma_start(out=wout_d[b0:b0 + BW, t, :],
                                          in_=w_sb)

                        # wT (bf16) for gates
                        wT_ps = ptr.tile([N, BW], f32, tag="tr")
                        nc.tensor.transpose(wT_ps, w_sb, ident_sb[0:BW, 0:BW])
                        wT_bf = smp.tile([N, BW], bf16, tag=f"wT_{w}")
                        nc.vector.tensor_copy(wT_bf, wT_ps)

                        # gates = wT.T@Wih + h2T.T@(Whh/2) + bias  [BW, 1024]
                        g_ps = pg.tile([BW, G4], f32, tag="g")
                        for q in range(0, G4, 512):
                            nc.tensor.matmul(g_ps[:, q:q + 512], lhsT=wT_bf,
                                             rhs=wih_sb[:, q:q + 512],
                                             start=True, stop=False)
                            nc.tensor.matmul(g_ps[:, q:q + 512],
                                             lhsT=st_["hT0"],
                                             rhs=whh_sb[:, q:q + 512],
                                             start=False, stop=False)
                            nc.tensor.matmul(g_ps[:, q:q + 512],
                                             lhsT=st_["hT1"],
                                             rhs=whh_sb[:, G4 + q:G4 + q + 512],
                                             start=False, stop=False)
                            nc.tensor.matmul(g_ps[:, q:q + 512], lhsT=ones_sb,
                                             rhs=brow_sb[:, q:q + 512],
                                             start=False, stop=True)

                        # LSTM with tanh-only nonlinearity (cols: i,f,o,g)
                        Tifo = smp.tile([BW, 3 * H], f32, tag=f"Tifo_{w}")
                        nc.scalar.activation(Tifo, g_ps[:, 0:3 * H], AF.Tanh,
                                             scale=0.5)
                        Tg = smp.tile([BW, H], f32, tag=f"Tg_{w}")
                        nc.scalar.activation(Tg, g_ps[:, 3 * H:4 * H], AF.Tanh)

                        u_sb = smp.tile([BW, H], f32, tag=f"u_{w}")
                        nc.vector.scalar_tensor_tensor(
                            out=u_sb, in0=Tifo[:, 0:H], scalar=1.0, in1=Tg,
                            op0=ALU.add, op1=ALU.mult)
                        v_sb = smp.tile([BW, H], f32, tag=f"v_{w}")
                        nc.vector.scalar_tensor_tensor(
                            out=v_sb, in0=Tifo[:, H:2 * H], scalar=1.0,
                            in1=st_["Ch"], op0=ALU.add, op1=ALU.mult)
                        c_sb = smp.tile([BW, H], f32, tag=f"c_{w}")
                        nc.vector.scalar_tensor_tensor(
                            out=c_sb, in0=u_sb, scalar=0.5, in1=v_sb,
                            op0=ALU.mult, op1=ALU.add)
                        Ch_new = stp.tile([BW, H], f32, tag=f"Ch_{w}")
                        nc.vector.tensor_scalar_mul(Ch_new, c_sb, 0.5)
                        Tc = smp.tile([BW, H], f32, tag=f"Tc_{w}")
                        nc.scalar.activation(Tc, c_sb, AF.Tanh)
                        h2_sb = smp.tile([BW, H], f32, tag=f"h2_{w}")
                        nc.vector.scalar_tensor_tensor(
                            out=h2_sb, in0=Tifo[:, 2 * H:3 * H], scalar=1.0,
                            in1=Tc, op0=ALU.add, op1=ALU.mult)
                        h_sb = outp.tile([BW, H], f32, tag=f"h_{w}")
                        nc.vector.tensor_scalar_mul(h_sb, h2_sb, 0.5)
                        nc.sync.dma_start(out=hout_d[b0:b0 + BW, t, :],
                                          in_=h_sb)

                        # transposes for next step state (h2T, cT) in bf16
                        new_st = {"Ch": Ch_new}
                        for nm, src in (("hT0", h2_sb[:, 0:128]),
                                        ("hT1", h2_sb[:, 128:256]),
                                        ("cT0", c_sb[:, 0:128]),
                                        ("cT1", c_sb[:, 128:256])):
                            tr_ps = ptr.tile([128, BW], f32, tag="tr")
                            nc.tensor.transpose(tr_ps, src,
                                                ident_sb[0:BW, 0:BW])
                            tr_bf = stp.tile([128, BW], bf16, tag=f"{nm}_{w}")
                            nc.vector.tensor_copy(tr_bf, tr_ps)
                            new_st[nm] = tr_bf
                        state[w] = new_st

    nc.finalize()
    return nc


def _prep_host_inputs(inputs):
    import ml_dtypes
    bf = ml_dtypes.bfloat16
    W1, b1 = np.asarray(inputs["W1"]), np.asarray(inputs["b1"])
    W2, b2 = np.asarray(inputs["W2"]), np.asarray(inputs["b2"])
    W3 = np.asarray(inputs["W3"])
    W_ih, W_hh = np.asarray(inputs["W_ih"]), np.asarray(inputs["W_hh"])
    b_ih, b_hh = np.asarray(inputs["b_ih"]), np.asarray(inputs["b_hh"])

    perm = _gate_perm()
    # z1 uses [h2=2h, c] so scale W1's h-columns by 0.5
    W1s = W1.copy()
    W1s[:, 0:H] *= 0.5  # columns 0:H=256 are the h dims of [h, c]
    W1T = W1s.T.astype(np.float32)          # [512, 99]
    w1t = W1T.reshape(4, 128, TT).transpose(1, 0, 2).reshape(128, 4 * TT)

    whh_half = (0.5 * W_hh.T)[:, perm]      # [256, 1024]
    whh = whh_half.reshape(2, 128, G4).transpose(1, 0, 2).reshape(128, 2 * G4)

    return {
        "w1t": w1t.astype(bf),
        "w2t": W2.T.astype(np.float32).astype(bf),      # [t, s] = W2[s,t].T
        "w3": W3[0].reshape(TT, 1).astype(bf),
        "biass": (b1 + b2).reshape(TT, 1).astype(np.float32),
        "wih": W_ih.T[:, perm].astype(bf),              # [128, 1024]
        "whh": whh.astype(bf),
        "brow": (b_ih + b_hh)[perm].reshape(1, G4).astype(bf),
        "ones1": np.ones((1, BW), dtype=bf),
        "ident": np.eye(128, dtype=np.float32),
    }


class _Runner:
    """Compile the Bass program into one reusable sharded PJRT executable."""

    def __init__(self, nc):
        import jax
        import concourse.mybir as mybir
        from concourse import bass2jax as b2j

        b2j.install_neuronx_cc_hook()
        self.jax = jax
        self.nc = nc

        part_name = (nc.partition_id_tensor.name
                     if nc.partition_id_tensor is not None else None)
        in_names, out_names, out_avals = [], [], []
        for alloc in nc.m.functions[0].allocations:
            if not isinstance(alloc, mybir.MemoryLocationSet):
                continue
            name = alloc.memorylocations[0].name
            if alloc.kind == "ExternalInput":
                if name != part_name:
                    in_names.append(name)
            elif alloc.kind == "ExternalOutput":
                out_names.append(name)
                out_avals.append(jax.core.ShapedArray(
                    tuple(alloc.tensor_shape), mybir.dt.np(alloc.dtype)))
        self.in_names, self.out_names, self.out_avals = in_names, out_names, out_avals
        n_params, n_outs = len(in_names), len(out_names)
        all_in_names = in_names + out_names
        if part_name is not None:
            all_in_names = all_in_names + [part_name]
        donate = tuple(range(n_params, n_params + n_outs))

        def _body(*args):
            operands = list(args)
            if part_name is not None:
                operands.append(b2j.partition_id_tensor())
            outs = b2j._bass_exec_p.bind(
                *operands,
                out_avals=tuple(out_avals),
                in_names=tuple(all_in_names),
                out_names=tuple(out_names),
                lowering_input_output_aliases=(),
                sim_require_finite=True,
                sim_require_nnan=True,
                nc=nc,
            )
            return tuple(outs)

        from jax.sharding import Mesh, PartitionSpec, NamedSharding
        from jax.experimental.shard_map import shard_map

        devices = jax.devices()[:NCORES]
        self.mesh = Mesh(np.asarray(devices), ("core",))
        self.sharding = NamedSharding(self.mesh, PartitionSpec("core"))
        in_specs = (PartitionSpec("core"),) * (n_params + n_outs)
        out_specs = (PartitionSpec("core"),) * n_outs
        self.fn = jax.jit(
            shard_map(_body, mesh=self.mesh, in_specs=in_specs,
                      out_specs=out_specs, check_rep=False),
            donate_argnums=donate, keep_unused=True)

    def _concat_inputs(self, in_maps):
        return [np.concatenate([np.asarray(m[name]) for m in in_maps], axis=0)
                for name in self.in_names]

    def _zero_outs(self):
        return [np.zeros((NCORES * a.shape[0], *a.shape[1:]), a.dtype)
                for a in self.out_avals]

    def run(self, in_maps, iters=1):
        jax = self.jax
        xs = [jax.device_put(a, self.sharding)
              for a in self._concat_inputs(in_maps)]
        outs = [jax.device_put(z, self.sharding) for z in self._zero_outs()]
        for _ in range(iters):
            outs = self.fn(*xs, *outs)
        jax.block_until_ready(outs)
        return {name: np.asarray(outs[i])
                for i, name in enumerate(self.out_names)}

    def measure(self, in_maps, k1=4, k2=24):
        """Per-execution wall time via slope between k1 and k2 chained runs."""
        import time as _time
        jax = self.jax
        xs = [jax.device_put(a, self.sharding)
              for a in self._concat_inputs(in_maps)]

        def loop(k):
            outs = [jax.device_put(z, self.sharding) for z in self._zero_outs()]
            jax.block_until_ready(outs); jax.block_until_ready(xs)
            t0 = _time.perf_counter()
            for _ in range(k):
                outs = self.fn(*xs, *outs)
            jax.block_until_ready(outs)
            return _time.perf_counter() - t0

        loop(2)  # warm
        t1 = min(loop(k1) for _ in range(3))
        t2 = min(loop(k2) for _ in range(3))
        return (t2 - t1) / (k2 - k1)


def _get_runner():
    if "runner" not in _cache:
        if "nc" not in _cache:
            _cache["nc"] = _build_program()
        _cache["runner"] = _Runner(_cache["nc"])
    return _cache["runner"]


def _make_in_maps(inputs):
    shared = _prep_host_inputs(inputs)
    x = np.ascontiguousarray(np.asarray(inputs["input_data"], dtype=np.float32))
    in_maps = []
    for c in range(NCORES):
        m = dict(shared)
        m["x"] = x[c * BSH:(c + 1) * BSH]
        in_maps.append(m)
    return in_maps


def kernel(**inputs):
    runner = _get_runner()
    res = runner.run(_make_in_maps(inputs))
    w_full = res["wout"].reshape(NCORES, BSH, TT, N).reshape(B_FULL, TT, N)
    h_full = res["hout"].reshape(NCORES, BSH, TT, H).reshape(B_FULL, TT, H)
    return w_full, h_full


# revision 7
# speedup vs baseline: 7838.6329x; 1.6267x over previous
"""Trainium2 Bass kernel for nn_Encoder (attention-augmented LSTM encoder).

Strategy: data-parallel over batch across 8 NeuronCores (64 rows each).
Per core the T-1=99 sequential recurrence runs with 2 software-pipelined
"waves" of 32 batch rows so all engines stay busy despite the serial
dependence. The big per-step tensor tanh(z1[b,s] + z2[b,n,s]) lives as
[s=99 partitions, (b,n) free] in bf16; its s-contraction with w3 runs as
32 per-batch-row stationary matmuls on the PE producing e^T across 128
partitions. Sigmoids are computed as 0.5*(1+tanh(x/2)) so the whole
kernel needs a single ACT table set (exp_and_others: exp + tanh).
"""
import sys
import numpy as np

sys.path.insert(0, "/opt/trn_rl_repo")

B_FULL, TT, N, H = 512, 99, 128, 256
NCORES = 8
BSH = B_FULL // NCORES          # 64 batch rows per core
WAVES = 2
BW = BSH // WAVES               # 32 rows per wave
G4 = 4 * H                      # 1024

_cache = {}


def _gate_perm():
    # pytorch gate order i,f,g,o -> our column order [i, f, o, g]
    idx = np.arange(G4)
    return np.concatenate([idx[0:H], idx[H:2*H], idx[3*H:4*H], idx[2*H:3*H]])


def _build_program(TT_run=TT):
    import concourse.bacc as bacc
    import concourse.mybir as mybir
    from concourse.tile import TileContext

    dt = mybir.dt
    AF = mybir.ActivationFunctionType
    ALU = mybir.AluOpType

    nc = bacc.Bacc("TRN2", target_bir_lowering=False, debug=False,
                   num_devices=NCORES)

    f32, bf16 = dt.float32, dt.bfloat16

    x_d = nc.dram_tensor("x", [BSH, TT, N], f32, kind="ExternalInput")
    w1t_d = nc.dram_tensor("w1t", [128, 4 * TT], bf16, kind="ExternalInput")
    w2t_d = nc.dram_tensor("w2t", [TT, TT], bf16, kind="ExternalInput")
    w3_d = nc.dram_tensor("w3", [TT, 1], bf16, kind="ExternalInput")
    biass_d = nc.dram_tensor("biass", [TT, 1], f32, kind="ExternalInput")
    wih_d = nc.dram_tensor("wih", [N, G4], bf16, kind="ExternalInput")
    whh_d = nc.dram_tensor("whh", [128, 2 * G4], bf16, kind="ExternalInput")
    brow_d = nc.dram_tensor("brow", [1, G4], bf16, kind="ExternalInput")
    ones_d = nc.dram_tensor("ones1", [1, BW], bf16, kind="ExternalInput")
    ident_d = nc.dram_tensor("ident", [128, 128], f32, kind="ExternalInput")
    wout_d = nc.dram_tensor("wout", [BSH, TT, N], f32, kind="ExternalOutput")
    hout_d = nc.dram_tensor("hout", [BSH, TT, H], f32, kind="ExternalOutput")

    with TileContext(nc) as tc:
        with tc.tile_pool(name="const", bufs=1) as cp:
            w1t_sb = cp.tile([128, 4 * TT], bf16)
            nc.sync.dma_start(out=w1t_sb, in_=w1t_d[:])
            w2t_sb = cp.tile([TT, TT], bf16)
            nc.sync.dma_start(out=w2t_sb, in_=w2t_d[:])
            w3_sb = cp.tile([TT, 1], bf16)
            nc.sync.dma_start(out=w3_sb, in_=w3_d[:])
            biass_sb = cp.tile([TT, 1], f32)
            nc.sync.dma_start(out=biass_sb, in_=biass_d[:])
            wih_sb = cp.tile([N, G4], bf16)
            nc.sync.dma_start(out=wih_sb, in_=wih_d[:])
            whh_sb = cp.tile([128, 2 * G4], bf16)
            nc.sync.dma_start(out=whh_sb, in_=whh_d[:])
            brow_sb = cp.tile([1, G4], bf16)
            nc.sync.dma_start(out=brow_sb, in_=brow_d[:])
            ones_sb = cp.tile([1, BW], bf16)
            nc.sync.dma_start(out=ones_sb, in_=ones_d[:])
            ident_sb = cp.tile([128, 128], f32)
            nc.sync.dma_start(out=ident_sb, in_=ident_d[:])
            z2_sb = cp.tile([TT, BSH * N], bf16)  # [s, b, n] b-outer

            # ---- z2 precompute: z2[s,b,n] = sum_t x[b,t,n] W2[s,t] + b1[s]+b2[s]
            with tc.tile_pool(name="pre", bufs=2) as pp, \
                 tc.tile_pool(name="prep", bufs=2, space="PSUM") as ppp:
                xT_sb = pp.tile([TT, BSH * N], f32, bufs=1)
                nc.sync.dma_start(out=xT_sb,
                                  in_=x_d[:].rearrange("b t n -> t b n"))
                xT_bf = pp.tile([TT, BSH * N], bf16, bufs=1)
                nc.vector.tensor_copy(xT_bf, xT_sb)
                # moving operand iterated (n-outer, b-inner) so z2 lands [s, n, b]
                xv = xT_bf.rearrange("t (b n) -> t n b", b=BSH)
                CH = 2048
                for c in range(0, BSH * N, CH):
                    zc_ps = ppp.tile([TT, CH], f32, tag="zc")
                    for cc in range(0, CH, 512):
                        n0 = (c + cc) // BSH
                        nc.tensor.matmul(zc_ps[:, cc:cc + 512],
                                         lhsT=w2t_sb,
                                         rhs=xv[:, n0:n0 + 512 // BSH, :],
                                         start=True, stop=True)
                    nc.scalar.activation(z2_sb[:, c:c + CH], zc_ps,
                                         AF.Identity, bias=biass_sb)

            # ---- main recurrence
            with tc.tile_pool(name="big", bufs=2) as bigp, \
                 tc.tile_pool(name="sm", bufs=3) as smp, \
                 tc.tile_pool(name="st", bufs=2) as stp, \
                 tc.tile_pool(name="xtp", bufs=6) as xtp, \
                 tc.tile_pool(name="outp", bufs=3) as outp, \
                 tc.tile_pool(name="pz1", bufs=1, space="PSUM") as pz1, \
                 tc.tile_pool(name="peT", bufs=1, space="PSUM") as peT, \
                 tc.tile_pool(name="pe", bufs=1, space="PSUM") as pe, \
                 tc.tile_pool(name="ptr", bufs=3, space="PSUM") as ptr, \
                 tc.tile_pool(name="pg", bufs=1, space="PSUM") as pg:

                # initial zero state per wave
                state = []
                for w in range(WAVES):
                    hT0 = stp.tile([128, BW], bf16, tag=f"hT0_{w}")
                    hT1 = stp.tile([128, BW], bf16, tag=f"hT1_{w}")
                    cT0 = stp.tile([128, BW], bf16, tag=f"cT0_{w}")
                    cT1 = stp.tile([128, BW], bf16, tag=f"cT1_{w}")
                    Ch = stp.tile([BW, H], f32, tag=f"Ch_{w}")
                    for tl in (hT0, hT1, cT0, cT1, Ch):
                        nc.vector.memset(tl, 0.0)
                    state.append({"hT0": hT0, "hT1": hT1, "cT0": cT0,
                                  "cT1": cT1, "Ch": Ch})

                z2v = z2_sb.rearrange("s (n b) -> s n b", b=BSH)

                for t in range(TT_run):
                    for w in range(WAVES):
                        st_ = state[w]
                        b0 = w * BW
                        # x_t prefetch
                        xt_sb = xtp.tile([BW, N], f32, tag=f"xt_{w}")
                        nc.sync.dma_start(out=xt_sb,
                                          in_=x_d[b0:b0 + BW, t, :])

                        # z1T = W1T.T @ [h2T; cT]  -> psum [99, BW]
                        z1_ps = pz1.tile([TT, BW], f32, tag="z1")
                        hc = (st_["hT0"], st_["hT1"], st_["cT0"], st_["cT1"])
                        for k in range(4):
                            nc.tensor.matmul(
                                z1_ps, lhsT=w1t_sb[:, k * TT:(k + 1) * TT],
                                rhs=hc[k], start=(k == 0), stop=(k == 3))
                        z1_bf = smp.tile([TT, BW], bf16, tag=f"z1bf_{w}")
                        nc.vector.tensor_copy(z1_bf, z1_ps)

                        # A = z2[:, wave, :] + bcast_n(z1)  (bf16)
                        A_bf = bigp.tile([TT, BW * N], bf16, tag=f"A_{w}")
                        nc.vector.tensor_tensor(
                            out=A_bf.rearrange("s (n b) -> s n b", b=BW),
                            in0=z2v[:, :, b0:b0 + BW],
                            in1=z1_bf.unsqueeze(1).broadcast_to([TT, N, BW]),
                            op=ALU.add)

                        # X = tanh(A)
                        X_bf = bigp.tile([TT, BW * N], bf16, tag=f"X_{w}")
                        nc.scalar.activation(X_bf, A_bf, AF.Tanh)

                        # eT[n, b] = sum_s w3[s] X[s, b, n]
                        eT_ps = peT.tile([N, BW], f32, tag="eT")
                        Xv = X_bf.rearrange("s (n b) -> s n b", b=BW)
                        for b in range(BW):
                            nc.tensor.matmul(eT_ps[:, b:b + 1],
                                             lhsT=Xv[:, :, b], rhs=w3_sb,
                                             start=True, stop=True)
                        eT_sb = smp.tile([N, BW], f32, tag=f"eT_{w}")
                        nc.vector.tensor_copy(eT_sb, eT_ps)

                        # e = transpose(eT)
                        e_ps = pe.tile([BW, N], f32, tag="e")
                        nc.tensor.transpose(e_ps, eT_sb, ident_sb)

                        # softmax (no max subtraction; |e| <~ 4) + x_t mult
                        P_sb = smp.tile([BW, N], f32, tag=f"P_{w}")
                        S_sb = smp.tile([BW, 1], f32, tag=f"S_{w}")
                        nc.scalar.activation(P_sb, e_ps, AF.Exp,
                                             accum_out=S_sb)
                        r_sb = smp.tile([BW, 1], f32, tag=f"r_{w}")
                        nc.vector.reciprocal(r_sb, S_sb)
                        w_sb = outp.tile([BW, N], f32, tag=f"w_{w}")
                        nc.vector.scalar_tensor_tensor(
                            out=w_sb, in0=P_sb, scalar=r_sb, in1=xt_sb,
                            op0=ALU.mult, op1=ALU.mult)
                        nc.sync.dma_start(out=wout_d[b0:b0 + BW, t, :],
                                          in_=w_sb)

                        # wT (bf16) for gates
                        wT_ps = ptr.tile([N, BW], f32, tag="tr")
                        nc.tensor.transpose(wT_ps, w_sb, ident_sb[0:BW, 0:BW])
                        wT_bf = smp.tile([N, BW], bf16, tag=f"wT_{w}")
                        nc.vector.tensor_copy(wT_bf, wT_ps)

                        # gates = wT.T@Wih + h2T.T@(Whh/2) + bias  [BW, 1024]
                        g_ps = pg.tile([BW, G4], f32, tag="g")
                        for q in range(0, G4, 512):
                            nc.tensor.matmul(g_ps[:, q:q + 512], lhsT=wT_bf,
                                             rhs=wih_sb[:, q:q + 512],
                                             start=True, stop=False)
                            nc.tensor.matmul(g_ps[:, q:q + 512],
                                             lhsT=st_["hT0"],
                                             rhs=whh_sb[:, q:q + 512],
                                             start=False, stop=False)
                            nc.tensor.matmul(g_ps[:, q:q + 512],
                                             lhsT=st_["hT1"],
                                             rhs=whh_sb[:, G4 + q:G4 + q + 512],
                                             start=False, stop=False)
                            nc.tensor.matmul(g_ps[:, q:q + 512], lhsT=ones_sb,
                                             rhs=brow_sb[:, q:q + 512],
                                             start=False, stop=True)

                        # LSTM, tanh-only (cols: i,f,o,g; g pre-scaled 2x
                        # host-side so one tanh(0.5*gates) covers all gates)
                        Tall = smp.tile([BW, G4], f32, tag=f"Tall_{w}")
                        nc.scalar.activation(Tall, g_ps, AF.Tanh, scale=0.5)

                        u_sb = smp.tile([BW, H], f32, tag=f"u_{w}")
                        nc.vector.scalar_tensor_tensor(
                            out=u_sb, in0=Tall[:, 0:H], scalar=1.0,
                            in1=Tall[:, 3 * H:4 * H],
                            op0=ALU.add, op1=ALU.mult)
                        v_sb = smp.tile([BW, H], f32, tag=f"v_{w}")
                        nc.vector.scalar_tensor_tensor(
                            out=v_sb, in0=Tall[:, H:2 * H], scalar=1.0,
                            in1=st_["Ch"], op0=ALU.add, op1=ALU.mult)
                        c_sb = smp.tile([BW, H], f32, tag=f"c_{w}")
                        nc.vector.scalar_tensor_tensor(
                            out=c_sb, in0=u_sb, scalar=0.5, in1=v_sb,
                            op0=ALU.mult, op1=ALU.add)
                        Ch_new = stp.tile([BW, H], f32, tag=f"Ch_{w}")
                        nc.vector.tensor_scalar_mul(Ch_new, c_sb, 0.5)
                        Tc = smp.tile([BW, H], f32, tag=f"Tc_{w}")
                        nc.scalar.activation(Tc, c_sb, AF.Tanh)
                        h2_sb = smp.tile([BW, H], f32, tag=f"h2_{w}")
                        nc.vector.scalar_tensor_tensor(
                            out=h2_sb, in0=Tall[:, 2 * H:3 * H], scalar=1.0,
                            in1=Tc, op0=ALU.add, op1=ALU.mult)
                        h_sb = outp.tile([BW, H], f32, tag=f"h_{w}")
                        nc.vector.tensor_scalar_mul(h_sb, h2_sb, 0.5)
                        nc.sync.dma_start(out=hout_d[b0:b0 + BW, t, :],
                                          in_=h_sb)

                        # transposes for next step state (h2T, cT) in bf16
                        new_st = {"Ch": Ch_new}
                        for nm, src in (("hT0", h2_sb[:, 0:128]),
                                        ("hT1", h2_sb[:, 128:256]),
                                        ("cT0", c_sb[:, 0:128]),
                                        ("cT1", c_sb[:, 128:256])):
                            tr_ps = ptr.tile([128, BW], f32, tag="tr")
                            nc.tensor.transpose(tr_ps, src,
                                                ident_sb[0:BW, 0:BW])
                            tr_bf = stp.tile([128, BW], bf16, tag=f"{nm}_{w}")
                            nc.vector.tensor_copy(tr_bf, tr_ps)
                            new_st[nm] = tr_bf
                        state[w] = new_st

    nc.finalize()
    return nc


def _prep_host_inputs(inputs):
    import ml_dtypes
    bf = ml_dtypes.bfloat16
    W1, b1 = np.asarray(inputs["W1"]), np.asarray(inputs["b1"])
    W2, b2 = np.asarray(inputs["W2"]), np.asarray(inputs["b2"])
    W3 = np.asarray(inputs["W3"])
    W_ih, W_hh = np.asarray(inputs["W_ih"]), np.asarray(inputs["W_hh"])
    b_ih, b_hh = np.asarray(inputs["b_ih"]), np.asarray(inputs["b_hh"])

    perm = _gate_perm()
    # z1 uses [h2=2h, c] so scale W1's h-columns by 0.5
    W1s = W1.copy()
    W1s[:, 0:H] *= 0.5  # columns 0:H=256 are the h dims of [h, c]
    W1T = W1s.T.astype(np.float32)          # [512, 99]
    w1t = W1T.reshape(4, 128, TT).transpose(1, 0, 2).reshape(128, 4 * TT)

    whh_half = (0.5 * W_hh.T)[:, perm]      # [256, 1024]
    whh = whh_half.reshape(2, 128, G4).transpose(1, 0, 2).reshape(128, 2 * G4)

    gsl = slice(3 * H, 4 * H)  # g-gate columns after permutation
    wih_p = W_ih.T[:, perm].astype(np.float32)
    wih_p[:, gsl] *= 2.0
    whh_p2 = whh_half.astype(np.float32).copy()
    whh_p2[:, gsl] *= 2.0
    whh = whh_p2.reshape(2, 128, G4).transpose(1, 0, 2).reshape(128, 2 * G4)
    brow_p = (b_ih + b_hh)[perm].astype(np.float32)
    brow_p[gsl] *= 2.0
    return {
        "w1t": w1t.astype(bf),
        "w2t": W2.T.astype(np.float32).astype(bf),      # [t, s] = W2[s,t].T
        "w3": W3[0].reshape(TT, 1).astype(bf),
        "biass": (b1 + b2).reshape(TT, 1).astype(np.float32),
        "wih": wih_p.astype(bf),                        # [128, 1024]
        "whh": whh.astype(bf),
        "brow": brow_p.reshape(1, G4).astype(bf),
        "ones1": np.ones((1, BW), dtype=bf),
        "ident": np.eye(128, dtype=np.float32),
    }


class _Runner:
    """Compile the Bass program into one reusable sharded PJRT executable."""

    def __init__(self, nc):
        import jax
        import concourse.mybir as mybir
        from concourse import bass2jax as b2j

        b2j.install_neuronx_cc_hook()
        self.jax = jax
        self.nc = nc

        part_name = (nc.partition_id_tensor.name
                     if nc.partition_id_tensor is not None else None)
        in_names, out_names, out_avals = [], [], []
        for alloc in nc.m.functions[0].allocations:
            if not isinstance(alloc, mybir.MemoryLocationSet):
                continue
            name = alloc.memorylocations[0].name
            if alloc.kind == "ExternalInput":
                if name != part_name:
                    in_names.append(name)
            elif alloc.kind == "ExternalOutput":
                out_names.append(name)
                out_avals.append(jax.core.ShapedArray(
                    tuple(alloc.tensor_shape), mybir.dt.np(alloc.dtype)))
        self.in_names, self.out_names, self.out_avals = in_names, out_names, out_avals
        n_params, n_outs = len(in_names), len(out_names)
        all_in_names = in_names + out_names
        if part_name is not None:
            all_in_names = all_in_names + [part_name]
        donate = tuple(range(n_params, n_params + n_outs))

        def _body(*args):
            operands = list(args)
            if part_name is not None:
                operands.append(b2j.partition_id_tensor())
            outs = b2j._bass_exec_p.bind(
                *operands,
                out_avals=tuple(out_avals),
                in_names=tuple(all_in_names),
                out_names=tuple(out_names),
                lowering_input_output_aliases=(),
                sim_require_finite=True,
                sim_require_nnan=True,
                nc=nc,
            )
            return tuple(outs)

        from jax.sharding import Mesh, PartitionSpec, NamedSharding
        from jax.experimental.shard_map import shard_map

        devices = jax.devices()[:NCORES]
        self.mesh = Mesh(np.asarray(devices), ("core",))
        self.sharding = NamedSharding(self.mesh, PartitionSpec("core"))
        in_specs = (PartitionSpec("core"),) * (n_params + n_outs)
        out_specs = (PartitionSpec("core"),) * n_outs
        self.fn = jax.jit(
            shard_map(_body, mesh=self.mesh, in_specs=in_specs,
                      out_specs=out_specs, check_rep=False),
            donate_argnums=donate, keep_unused=True)

    def _concat_inputs(self, in_maps):
        return [np.concatenate([np.asarray(m[name]) for m in in_maps], axis=0)
                for name in self.in_names]

    def _zero_outs(self):
        return [np.zeros((NCORES * a.shape[0], *a.shape[1:]), a.dtype)
                for a in self.out_avals]

    def run(self, in_maps, iters=1):
        jax = self.jax
        xs = [jax.device_put(a, self.sharding)
              for a in self._concat_inputs(in_maps)]
        outs = [jax.device_put(z, self.sharding) for z in self._zero_outs()]
        for _ in range(iters):
            outs = self.fn(*xs, *outs)
        jax.block_until_ready(outs)
        return {name: np.asarray(outs[i])
                for i, name in enumerate(self.out_names)}

    def measure(self, in_maps, k1=4, k2=24):
        """Per-execution wall time via slope between k1 and k2 chained runs."""
        import time as _time
        jax = self.jax
        xs = [jax.device_put(a, self.sharding)
              for a in self._concat_inputs(in_maps)]

        def loop(k):
            outs = [jax.device_put(z, self.sharding) for z in self._zero_outs()]
            jax.block_until_ready(outs); jax.block_until_ready(xs)
            t0 = _time.perf_counter()
            for _ in range(k):
                outs = self.fn(*xs, *outs)
            jax.block_until_ready(outs)
            return _time.perf_counter() - t0

        loop(2)  # warm
        t1 = min(loop(k1) for _ in range(3))
        t2 = min(loop(k2) for _ in range(3))
        return (t2 - t1) / (k2 - k1)


def _get_runner():
    if "runner" not in _cache:
        if "nc" not in _cache:
            _cache["nc"] = _build_program()
        _cache["runner"] = _Runner(_cache["nc"])
    return _cache["runner"]


def _make_in_maps(inputs):
    shared = _prep_host_inputs(inputs)
    x = np.ascontiguousarray(np.asarray(inputs["input_data"], dtype=np.float32))
    in_maps = []
    for c in range(NCORES):
        m = dict(shared)
        m["x"] = x[c * BSH:(c + 1) * BSH]
        in_maps.append(m)
    return in_maps


def kernel(**inputs):
    runner = _get_runner()
    res = runner.run(_make_in_maps(inputs))
    w_full = res["wout"].reshape(NCORES, BSH, TT, N).reshape(B_FULL, TT, N)
    h_full = res["hout"].reshape(NCORES, BSH, TT, H).reshape(B_FULL, TT, H)
    return w_full, h_full
